# revision 38
# baseline (speedup 1.0000x reference)
"""ExpertsChooseMlp Trainium2 kernel.

Full inputs in, full output out. Sharding: 8 cores = 4 batches x 2 expert-pairs.
Core m handles batch b=m//2 and experts {2g, 2g+1}, g=m%2. Each core computes
pout[T,O] = sum_{e in pair} combine[b,:,e,:] @ mlp_e(dispatch[b,:,e,:]^T @ x[b]);
the host sums the two partials per batch, applies the w2 rank-1 correction
and adds b2.

Precision: every matmul operand in fp8-e4m3 (TRN variant: max +-240, values
above convert to Inf), accumulation in fp32 PSUM. All four contractions run
as fp8 DoubleRow matmuls (2 K-planes of 128 per pass) which issue at the
same ~215ns N=512 stream rate as bf16 -> 2x throughput, halving the matmul
count 512 -> 320. Host packs each K=256 block of the contraction dim as
[plane i][partition p] (row k = base + i*128 + p):
  xdT[D,C] = DRmm(lhsT=x[T2,i,D],   rhs=dm[T2,i,C])    (K=T,  8 passes)
  hT[HE,C] = DRmm(lhsT=w1[D2,i,HE], rhs=xdT[D2,i,C])   (K=D,  2 passes)
  y[C,O]   = DRmm(lhsT=hT[H2,i,C],  rhs=w2[H2,i,O])    (K=HE, 2 passes)
  pout[T,O]= DRmm(lhsT=cmT[C2,i,T], rhs=y[C2,i,O])     (K=C,  8 passes, +e)

fp8 error control: the output is dominated by a rank-1 "DC" component (the
masks have mean 0.5, gelu output has positive mean) which amplifies the
signal ~17x over generic per-element noise. Quantization error of x/w1/w2
rides that same DC path (their error column-sums are amplified); masks and
intermediates are not. Countermeasures, each killing the amplified term:
 - x: error-feedback quantization along T (quant-error prefix sums ~1 ulp).
 - w1: exact host bias fold b1 += mean_c(xd) @ (w1 - fp8(w1)); mean_c(xd)
   is host-computable from the quantized dm/x in O(T*(C+D)).
 - w2: rank-1 correction. The gelu ACTIVATE's accum_out yields
   Hsum_h = sum_c ht[h,c] for ~free; the tiny [P,4,2] sums are DMA'd out
   and the host adds outer(sum_c cm8, Hsum @ (w2 - fp8(w2)))/C.
Measured: max rel err 7.4e-3 (threshold 2e-2).

Schedule notes (from perfetto traces): MMs issue at the N=512 stream rate
(~215ns) with ~5us of residual gaps. 8 warm matmuls on a memset tile bridge
the ~4us from the engine barrier to the first DMA completion release so
real matmuls start at the HAM 2.4GHz clock. DMA rings advance ~1
descriptor/0.73us regardless of size, so the SYNC ring is ordered
dm-e0(pass-0 halves first) -> w1 -> w2 -> dm-e1 -> cmt -> outputs, and the
ACT ring carries x chunks + b1 + the (late, blocking-safe) hsum exports.
Fixed costs outside kernel control: ~4.5us bass preamble (const memsets,
act-table load, engine barrier) and ~10.5us NEFF teardown (per-engine
semaphore zeroing emitted by the compiler), both inside the measured span.
Measured HW exec: ~87.2us (baseline 128.9us).
"""
import sys

sys.path.insert(0, "/opt/trn_rl_repo")

import numpy as np
import ml_dtypes

import concourse.bacc as bacc
import concourse.mybir as mybir
import concourse.tile as tile
from concourse import bass_utils

B, T, D, E, C, HE, O = 4, 2048, 512, 4, 1024, 512, 512
P = 128
nKT2 = T // 256   # 8  T DR-chunks (K=256 each)
nMD = D // P      # 4  D-chunks
nMH = HE // P     # 4  HE-chunks
nKD2 = D // 256   # 2  D DR-chunks
nCC = C // P      # 8  C-chunks
nKH2 = HE // 256  # 2  HE DR-chunks
nMT = T // P      # 16
NF = 512          # matmul free dim (one PSUM bank)

F32 = mybir.dt.float32
BF16 = mybir.dt.bfloat16
F8 = mybir.dt.float8e4
GELU = mybir.ActivationFunctionType.Gelu
DR = mybir.MatmulPerfMode.DoubleRow
nCP = nCC // 2    # 4  C pair-chunks

_NC = None


def _build():
    nc = bacc.Bacc("TRN2", target_bir_lowering=False, debug=False,
                   enable_asserts=False, num_devices=1)
    xq = nc.dram_tensor("xq", [P, nKT2, 2, D], F8, kind="ExternalInput").ap()
    dmq = nc.dram_tensor("dmq", [2, nKT2, P, 2, 2, NF], F8,
                         kind="ExternalInput").ap()
    cmq = nc.dram_tensor("cmq", [2, nCP, P, 2, T], F8, kind="ExternalInput").ap()
    w1q = nc.dram_tensor("w1q", [P, 2, nKD2, 2, HE], F8, kind="ExternalInput").ap()
    w2q = nc.dram_tensor("w2q", [P, 2, nKH2, 2, O], F8, kind="ExternalInput").ap()
    b1 = nc.dram_tensor("b1s", [2, HE], F32, kind="ExternalInput").ap()
    pout = nc.dram_tensor("pout", [T, O], BF16, kind="ExternalOutput").ap()
    hsum = nc.dram_tensor("hsum", [2, P, nMH, 2], BF16, kind="ExternalOutput").ap()

    with tile.TileContext(nc) as tc:
        with (
            tc.tile_pool(name="const", bufs=1) as const,
            tc.tile_pool(name="dmp", bufs=16) as dmp,
            tc.tile_pool(name="cmp", bufs=8) as cmp_,
            tc.tile_pool(name="inter", bufs=1) as inter,
            tc.tile_pool(name="yp", bufs=2) as yp,
            tc.tile_pool(name="outp", bufs=4) as outp,
            tc.tile_pool(name="psum", bufs=8, space="PSUM") as psp,
        ):
            # ---- resident constants (ACT HWDGE ring) ----
            # x split per kt2-chunk so the first matmul isn't gated on a 1MB DMA
            x_sb = const.tile([P, nKT2, 2, D], F8)
            for kt in range(nKT2):
                nc.scalar.dma_start(x_sb[:, kt, :, :], xq[:, kt, :, :])
            b1_sb = const.tile([P, 2 * nMH], F32)
            nc.scalar.dma_start(b1_sb[:], b1.rearrange("e (mh p) -> p (e mh)", p=P))

            # expert-0 dispatch-mask tiles FIRST on the SYNC ring, split into
            # per-pass halves with ALL pass-0 halves ahead: phase A's first
            # accumulation pass then needs only half the mask bytes, doubling
            # the early supply slack (the ring advances ~1 descriptor/0.73us
            # regardless of size, so granularity is what buys headroom).
            dm_e0 = []
            for kt in range(nKT2):
                dm_e0.append(dmp.tile([P, 2, 2, NF], F8, tag="dm",
                                      name=f"dm0_{kt}"))
            for pn in range(2):
                for kt in range(nKT2):
                    nc.sync.dma_start(dm_e0[kt][:, pn, :, :], dmq[0, kt, :, pn])
            # weights ride the same ring BEHIND dm-e0 (FIFO): they are only
            # needed from phase B (~19us) onward.
            w1_sb = const.tile([P, 2, nKD2, 2, HE], F8)
            nc.sync.dma_start(w1_sb[:], w1q[:])
            w2_sb = const.tile([P, 2, nKH2, 2, O], F8)
            nc.sync.dma_start(w2_sb[:], w2q[:])

            # ---- HAM warmup: dummy matmuls on a memset tile while the first
            # dm/x DMAs land, so real matmuls start closer to 2.4GHz.
            warm = const.tile([P, NF], BF16)
            nc.gpsimd.memset(warm[:], 0.0)
            # 8 big warm matmuls bridge from the engine barrier toward the
            # first DMA batch's completion-semaphore release (~11.1-11.6us),
            # then a tail of tiny N=64 matmuls (~90ns each) keeps the PE busy
            # up to the release with minimal overshoot: any idle there breaks
            # the HAM busy-window and the first real matmuls run at 1.2GHz.
            ps_w = psp.tile([P, NF], F32, tag="ps", name="ps_warm")
            for i in range(8):
                nc.tensor.matmul(ps_w[:], warm[:, 0:P], warm[:],
                                 start=(i == 0), stop=False)
            for j in range(10):
                nc.tensor.matmul(ps_w[:, 0:64], warm[:, 0:P], warm[:, 0:64],
                                 start=False, stop=(j == 9))

            y_tiles = []
            for ei in range(2):
                # ---- dispatch-mask tiles for this expert (SYNC ring) ----
                if ei == 0:
                    dm_t = dm_e0
                else:
                    dm_t = []
                    for kt in range(nKT2):
                        t_ = dmp.tile([P, 2, 2, NF], F8, tag="dm")
                        nc.sync.dma_start(t_[:], dmq[ei, kt])
                        dm_t.append(t_)

                # ---- phase A: xdT[D, C] (fp8 DR over K=T) ----
                # ncc-split: finish the 4 ncc=0 banks first so their PSUM->SBUF
                # evacuations overlap the ncc=1 accumulation pass and phase B
                # never starves on xdt copies.
                xdt = inter.tile([P, nKD2, 2, C], F8, tag="xdt")
                pss = [psp.tile([P, NF], F32, tag="ps", name=f"psa{i}")
                       for i in range(2 * nMD)]
                for pn in range(2):
                    for kt in range(nKT2):
                        for mc in range(nMD):
                            nc.tensor.matmul(pss[2 * mc + pn][:],
                                             x_sb[:, kt, :, mc * P:(mc + 1) * P],
                                             dm_t[kt][:, pn, :, :],
                                             start=(kt == 0), stop=(kt == nKT2 - 1),
                                             perf_mode=DR)
                    # D-row d = mc*128 + p = (mc//2)*256 + (mc%2)*128 + p.
                    # All copies on DVE: ScalarE must stay GELU-only or it
                    # reloads the activation table (1.3us) at every switch.
                    for mc in range(nMD):
                        nc.vector.tensor_copy(
                            xdt[:, mc // 2, mc % 2, pn * NF:(pn + 1) * NF],
                            pss[2 * mc + pn][:])

                # ---- phase B: hT[HE, C] = gelu(w1^T xdT + b1) (fp8 DR) ----
                # ncc-outer so phase C's first C-half unblocks after 4 gelus.
                # accum_out collects Hsum_h = sum_c ht[h, c] for the w2 corr.
                ht = inter.tile([P, nKH2, 2, C], F8, tag="ht")
                hs = inter.tile([P, nMH, 2], BF16, tag="hs")
                for ncc in range(2):
                    sl = slice(ncc * NF, (ncc + 1) * NF)
                    for mh in range(nMH):
                        ps0 = psp.tile([P, NF], F32, tag="ps")
                        for kd in range(nKD2):
                            nc.tensor.matmul(ps0[:],
                                             w1_sb[:, ei, kd, :, mh * P:(mh + 1) * P],
                                             xdt[:, kd, :, sl],
                                             start=(kd == 0), stop=(kd == nKD2 - 1),
                                             perf_mode=DR)
                        bia = b1_sb[:, ei * nMH + mh:ei * nMH + mh + 1]
                        # bf16 accum_out: it feeds the rank-1 correction whose
                        # own magnitude is ~2% of the output, so bf16's 0.4%
                        # is harmless — and the t-matmul stays single-pass
                        # (an fp32 matmul runs LOW_HIGH double-pass on PE).
                        with nc.allow_low_precision(reason="w2-corr accum"):
                            nc.scalar.activation(ht[:, mh // 2, mh % 2, sl],
                                                 ps0[:], GELU, bias=bia,
                                                 accum_out=hs[:, mh, ncc:ncc + 1])

                # ---- phase C: y[C, O] (fp8 DR; DR plane layout:
                # row c = kp*256 + i*128 + p  ->  y_sb[p, kp, i, :]) ----
                y_sb = yp.tile([P, nCP, 2, O], F8, tag="y")
                for cc in range(nCC):
                    ps = psp.tile([P, NF], F32, tag="ps")
                    for kh in range(nKH2):
                        nc.tensor.matmul(ps[:],
                                         ht[:, kh, :, cc * P:(cc + 1) * P],
                                         w2_sb[:, ei, kh, :, :],
                                         start=(kh == 0), stop=(kh == nKH2 - 1),
                                         perf_mode=DR)
                    nc.vector.tensor_copy(y_sb[:, cc // 2, cc % 2, :], ps[:])
                y_tiles.append(y_sb)

                # ---- w2 rank-1 correction: ship the tiny Hsum to the host,
                # which forms t = Hsum @ (w2 - fp8(w2)) and adds
                # outer(sum_c cm8, t)/C — zero PE cost. DMA rides the ACT
                # ring (idle; a waiting descriptor there can't block the
                # mask loads on the SYNC ring).
                nc.scalar.dma_start(hsum[ei], hs[:])

            # ---- combine-mask tiles (fp8, [P, plane, T]): SYNC ring behind
            # the dm loads so they can't steal early HBM bandwidth ----
            cmt_t = {}
            for ei in range(2):
                for kp in range(nCP):
                    t_ = cmp_.tile([P, 2, T], F8, tag="cmt")
                    nc.sync.dma_start(t_[:], cmq[ei, kp])
                    cmt_t[(ei, kp)] = t_

            # ---- phase D: pout[T, O] = sum_e cmT_e^T y_e (fp8 DR) ----
            for mt in range(nMT):
                ps = psp.tile([P, NF], F32, tag="ps")
                idx = 0
                for ei in range(2):
                    for kp in range(nCP):
                        nc.tensor.matmul(ps[:],
                                         cmt_t[(ei, kp)][:, :, mt * P:(mt + 1) * P],
                                         y_tiles[ei][:, kp, :, :],
                                         start=(idx == 0), stop=(idx == 7),
                                         perf_mode=DR)
                        idx += 1
                ot = outp.tile([P, O], BF16, tag="out")
                nc.vector.tensor_copy(ot[:], ps[:])
                nc.sync.dma_start(pout[mt * P:(mt + 1) * P, :], ot[:])

    nc.compile()
    return nc


def get_nc():
    global _NC
    if _NC is None:
        _NC = _build()
    return _NC


_F8 = ml_dtypes.float8_e4m3


def _qef(a):
    """fp8 quantization with error feedback along axis 0 (keeps running
    column sums of the quantization error bounded by ~1 ulp)."""
    out = np.empty(a.shape, _F8)
    carry = np.zeros(a.shape[1:], np.float32)
    for t in range(a.shape[0]):
        v = a[t] + carry
        q = v.astype(_F8)
        out[t] = q
        carry = v - q.astype(np.float32)
    return out


def make_in_maps(x, dispatch_mask, combine_array, w1, b1, w2):
    in_maps = []
    meta = []
    x8 = {}
    for b in range(B):
        x8[b] = _qef(x[b])                       # [T, D] fp8, EF along T
    for m in range(8):
        b, g = m // 2, m % 2
        es = slice(2 * g, 2 * g + 2)
        x8f = x8[b].astype(np.float32)
        xs = np.ascontiguousarray(
            x8[b].reshape(nKT2, 2, P, D).transpose(2, 0, 1, 3))
        # dm: [e, kt2, p, pn, i, c'], row t = kt2*256 + i*128 + p,
        # col c = pn*512 + c'
        dm_s = np.transpose(dispatch_mask[b, :, es, :], (1, 0, 2)).astype(_F8)
        dm_q = np.ascontiguousarray(
            dm_s.reshape(2, nKT2, 2, P, 2, NF).transpose(0, 1, 3, 4, 2, 5))
        # cmT: [e, kp, p, i, t], row c = kp*256 + i*128 + p
        cm_s = np.transpose(combine_array[b, :, es, :], (1, 2, 0)).astype(_F8)
        cm_q = np.ascontiguousarray(
            cm_s.reshape(2, nCP, 2, P, T).transpose(0, 1, 3, 2, 4))
        w1_8 = w1[es].astype(_F8)                # [2, D, HE]
        w2_8 = w2[es].astype(_F8)                # [2, HE, O]
        # w1 bias fold: b1' = b1 + mean_c(xd) @ (w1 - w18),
        # mean_c(xd)_d = sum_t mean_c(dm8[t,:]) * x8[t,d]
        b1c = np.empty((2, HE), np.float32)
        for e in range(2):
            rm = dm_s[e].astype(np.float32).mean(axis=1)      # [T]
            xbar = rm @ x8f                                   # [D]
            b1c[e] = b1[es][e] + xbar @ (w1[es][e] - w1_8[e].astype(np.float32))
        dw2_s = (w2[es] - w2_8.astype(np.float32))            # [2, HE, O]
        # host-side combine weights for the w2 correction
        cmsum = cm_s.astype(np.float32).sum(axis=1)           # [2, T]
        meta.append((cmsum, dw2_s))
        in_maps.append({
            "xq": xs,
            "dmq": dm_q,
            "cmq": cm_q,
            "w1q": np.ascontiguousarray(
                w1_8.reshape(2, nKD2, 2, P, HE).transpose(3, 0, 1, 2, 4)),
            "w2q": np.ascontiguousarray(
                w2_8.reshape(2, nKH2, 2, P, O).transpose(3, 0, 1, 2, 4)),
            "b1s": np.ascontiguousarray(b1c),
        })
    return in_maps, meta


def kernel(x, dispatch_mask, combine_array, w1, b1, w2, b2):
    nc = get_nc()
    x, dispatch_mask, combine_array, w1, b1, w2 = (
        np.asarray(a, dtype=np.float32)
        for a in (x, dispatch_mask, combine_array, w1, b1, w2))
    in_maps, meta = make_in_maps(x, dispatch_mask, combine_array, w1, b1, w2)
    res = bass_utils.run_bass_kernel_spmd(nc, in_maps, core_ids=list(range(8)))
    b2f = np.asarray(b2, dtype=np.float32)
    out = np.empty((B, T, O), dtype=np.float32)
    for b in range(B):
        acc = np.zeros((T, O), np.float32)
        for g in range(2):
            m = 2 * b + g
            acc += res.results[m]["pout"].astype(np.float32)
            hsv = res.results[m]["hsum"].astype(np.float32)   # [2, P, nMH, 2]
            cmsum, dw2_s = meta[m]                            # [2,T], [2,HE,O]
            for e in range(2):
                # Hsum[mh*128 + p] = sum over both C-halves
                hfull = hsv[e].sum(axis=2).T.reshape(HE)      # [HE]
                t_full = (hfull @ dw2_s[e]) * (1.0 / C)       # [O]
                acc += np.outer(cmsum[e], t_full)
        out[b] = acc + b2f
    return out
